# revision 1
# baseline (speedup 1.0000x reference)
"""Compact Bilinear Pooling (count-sketch + circular conv + spatial sum-pool)
as a Trainium2 Bass/Tile kernel, SPMD over 8 NeuronCores.

Math: with sk_i = flat @ S_i (flat: [B*P, C]), the reference computes
    out[b] = sum_{p in sample b} ifft( fft(sk1_p) * fft(sk2_p) ).real
Fold the (constant) sketch matrices into the DFT:  Phi_i = fft(S_i, axis=1),
so fft(sk_i,p) = x_p^T Phi_i.  Because sk are real, only the half spectrum
k = 0..D/2 is needed:
    Shat[b,k]  = sum_p (x_p^T Phi1[:,k]) * (x_p^T Phi2[:,k])
    out[b,d]   = sum_k wk*( Re Shat * cos(2 pi k d/D) - Im Shat * sin(...) )/D
Device pipeline per core (sharded by frequency: 512 of 4096 padded freqs):
  1. Ghat tiles [128 pos, 512 freq] via f32r matmuls (full PE rate).
  2. complex products on DVE; per-sample position-reduction via +-1
     indicator matmuls accumulating into one PSUM bank -> Shat [16, 512].
  3. inverse DFT as bf16 matmul against precomputed cos/sin slabs,
     partial [16, 8000] per core; host sums the 8 partials.
"""

import numpy as np
import ml_dtypes

import concourse.bacc as bacc
import concourse.mybir as mybir
import concourse.tile as tile
from concourse.bass_utils import run_bass_kernel_spmd

# problem dims (hardcoded per spec)
B, C, H, W, D = 16, 512, 14, 14, 8000
P = H * W            # 196 positions per sample
BP = B * P           # 3136
KH = D // 2 + 1      # 4001 half-spectrum frequencies
KPAD = 4096          # padded to 8*512
NCORES = 8
KSL = KPAD // NCORES  # 512 freqs per core
NCC = C // 128        # 4 contraction chunks
NPT = (BP + 127) // 128  # 25 position tiles (24x128 + 64)
DQ = D // 4           # 2000 output cols per quarter
DCH = 500             # inverse matmul free dim (4 chunks per quarter)
NKT = KSL // 128      # 4 k-chunks of the core's freq slice

F32R = mybir.dt.float32r
F32 = mybir.dt.float32
BF16 = mybir.dt.bfloat16


def build_nc():
    nc = bacc.Bacc("TRN2", target_bir_lowering=False, debug=False)
    x_d = nc.dram_tensor("x", [B, C, P], F32R, kind="ExternalInput")
    phi_d = nc.dram_tensor("phi", [128, NCC * 4 * KSL], F32R, kind="ExternalInput")
    ind_d = nc.dram_tensor("ind", [128, 3 * NPT * 2 * B], F32R, kind="ExternalInput")
    cc_d = nc.dram_tensor("cc", [2, NKT, 128, D], BF16, kind="ExternalInput")
    out_d = nc.dram_tensor("out", [B, D], F32, kind="ExternalOutput")

    xa = x_d.ap().rearrange("b c p -> c b p")  # [512, 16, 196]

    with tile.TileContext(nc) as tc:
        with (
            tc.tile_pool(name="phi", bufs=1) as phi_pool,
            tc.tile_pool(name="xin", bufs=1) as x_pool,
            tc.tile_pool(name="bcp", bufs=4) as b_pool,
            tc.tile_pool(name="prd", bufs=8) as prod_pool,
            tc.tile_pool(name="sbf", bufs=1) as s_pool,
            tc.tile_pool(name="cslab", bufs=16) as c_pool,
            tc.tile_pool(name="stage", bufs=2) as st_pool,
            tc.tile_pool(name="mm", bufs=5, space="PSUM") as mm_psum,
            tc.tile_pool(name="sac", bufs=1, space="PSUM") as s_psum,
            tc.tile_pool(name="inv", bufs=2, space="PSUM") as inv_psum,
        ):
            # ---- constants: phi (4 chunks), indicators, x (4 c-chunks)
            phit = phi_pool.tile([128, NCC * 4 * KSL], F32R)
            for i in range(4):
                sl = slice(i * 2048, (i + 1) * 2048)
                nc.sync.dma_start(phit[:, sl], phi_d.ap()[:, sl])
            indt = phi_pool.tile([128, 3 * NPT * 2 * B], F32R, tag="ind")
            nc.sync.dma_start(indt[:], ind_d.ap())

            xt = []
            for cci in range(NCC):
                t = x_pool.tile([128, B, P], F32R, tag=f"x{cci}")
                nc.sync.dma_start(t[:], xa[cci * 128:(cci + 1) * 128])
                xt.append(t[:].rearrange("c b p -> c (b p)"))

            # ---- Shat accumulator: rows 0:16 = Re, rows 16:32 = Im
            s_acc = s_psum.tile([32, KSL], F32, tag="sacc")

            # ---- main stage
            for pt in range(NPT):
                M = min(128, BP - pt * 128)
                ps = []
                for m in range(4):  # 0:g1re 1:g1im 2:g2re 3:g2im
                    g = mm_psum.tile([128, KSL], F32, tag="mm")
                    for cci in range(NCC):
                        nc.tensor.matmul(
                            g[0:M, :],
                            lhsT=xt[cci][:, pt * 128:pt * 128 + M],
                            rhs=phit[:, (cci * 4 + m) * KSL:(cci * 4 + m + 1) * KSL],
                            start=(cci == 0),
                            stop=(cci == NCC - 1),
                        )
                    ps.append(g)
                b2re = b_pool.tile([128, KSL], F32, tag="b2re")
                nc.scalar.copy(b2re[0:M, :], ps[2][0:M, :])
                b2im = b_pool.tile([128, KSL], F32, tag="b2im")
                nc.scalar.copy(b2im[0:M, :], ps[3][0:M, :])

                prods = []
                for in0, in1 in ((ps[0], b2re), (ps[1], b2im),
                                 (ps[0], b2im), (ps[1], b2re)):
                    pr = prod_pool.tile([128, KSL], F32R, tag="prod")
                    nc.vector.tensor_mul(pr[0:M, :], in0[0:M, :], in1[0:M, :])
                    prods.append(pr)

                # per-sample reduce over positions: S += ind^T @ prod
                # combined [M, 32] indicators: Re rows 0:16, Im rows 16:32
                # patterns: 0 = [+1|0] (RR), 1 = [-1|0] (II), 2 = [0|+1] (RI, IR)
                for i, (pr, pat) in enumerate((
                    (prods[0], 0), (prods[1], 1), (prods[2], 2), (prods[3], 2),
                )):
                    off = (pat * NPT + pt) * 2 * B
                    nc.tensor.matmul(
                        s_acc[:],
                        lhsT=indt[0:M, off:off + 2 * B],
                        rhs=pr[0:M, :],
                        start=(pt == 0 and i == 0),
                        stop=(pt == NPT - 1 and i == 3),
                        skip_group_check=True,
                    )

            # ---- Shat -> bf16, transpose to [freq, sample] via DMA transpose
            s_bf = s_pool.tile([32, KSL], BF16, tag="sbf")
            nc.scalar.copy(s_bf[:], s_acc[:])
            sT = []
            for kt in range(NKT):
                t = s_pool.tile([128, 32], BF16, tag=f"sT{kt}")
                nc.sync.dma_start(t[:], s_bf[:, kt * 128:(kt + 1) * 128],
                                  transpose=True)
                sT.append(t)

            # ---- inverse: out[b,d] = sum_k Sre*Cre + Sim*Cim  (bf16 matmuls)
            for q in range(4):
                slabs = {}
                for t in range(2):
                    for kt in range(NKT):
                        st = c_pool.tile([128, DQ], BF16, tag="cslab")
                        nc.sync.dma_start(
                            st[:], cc_d.ap()[t, kt, :, q * DQ:(q + 1) * DQ]
                        )
                        slabs[(t, kt)] = st
                stage = st_pool.tile([B, DQ], F32, tag="stage")
                for dq in range(4):
                    pinv = inv_psum.tile([B, DCH], F32, tag="inv")
                    idx = 0
                    for t, col in ((0, 0), (1, 16)):
                        for kt in range(NKT):
                            nc.tensor.matmul(
                                pinv[:],
                                lhsT=sT[kt][:, col:col + B],
                                rhs=slabs[(t, kt)][:, dq * DCH:(dq + 1) * DCH],
                                start=(idx == 0),
                                stop=(idx == 7),
                            )
                            idx += 1
                    nc.scalar.copy(stage[:, dq * DCH:(dq + 1) * DCH], pinv[:])
                nc.sync.dma_start(out_d.ap()[:, q * DQ:(q + 1) * DQ], stage[:])

    nc.compile()
    return nc


def make_constants(S1, S2):
    """Host-side constant prep from the sketch matrices (per-core slices)."""
    Phi = np.zeros((4, C, KPAD), np.float32)
    for i, S in enumerate((S1, S2)):
        F = np.fft.fft(S.astype(np.float64), axis=1)[:, :KH]
        Phi[2 * i, :, :KH] = F.real.astype(np.float32)
        Phi[2 * i + 1, :, :KH] = F.imag.astype(np.float32)

    k = np.arange(KPAD, dtype=np.float64)
    wk = np.where((k == 0) | (k == D // 2), 1.0, 2.0) / D
    wk[KH:] = 0.0
    ang = 2.0 * np.pi * np.outer(k, np.arange(D, dtype=np.float64)) / D
    Cst = np.stack([wk[:, None] * np.cos(ang), -wk[:, None] * np.sin(ang)])
    Cst = Cst.astype(ml_dtypes.bfloat16)  # [2, KPAD, D]

    # phi_packed[j]: [128, (cc, m, kk)] = Phi[m, cc*128+p, 512j+kk]
    arr = Phi.reshape(4, NCC, 128, NCORES, KSL)  # [m, cc, p, j, kk]
    phis, ccs = [], []
    for j in range(NCORES):
        a = arr[:, :, :, j]                      # [m, cc, p, kk]
        a = np.ascontiguousarray(np.transpose(a, (1, 0, 2, 3)))  # [cc, m, p, kk]
        phis.append(np.ascontiguousarray(
            a.transpose(2, 0, 1, 3).reshape(128, NCC * 4 * KSL)))
        c = Cst.reshape(2, NCORES, NKT, 128, D)[:, j]  # [2, kt, 128, D]
        ccs.append(np.ascontiguousarray(c))

    # indicators: [128, (pattern, pt, 2B)] with Re cols 0:16, Im cols 16:32
    # pattern 0 = [+1|0] (RR), 1 = [-1|0] (II), 2 = [0|+1] (RI, IR)
    ind = np.zeros((128, 3 * NPT * 2 * B), np.float32)
    for pt in range(NPT):
        for r in range(min(128, BP - pt * 128)):
            b = (pt * 128 + r) // P
            ind[r, (0 * NPT + pt) * 2 * B + b] = 1.0
            ind[r, (1 * NPT + pt) * 2 * B + b] = -1.0
            ind[r, (2 * NPT + pt) * 2 * B + B + b] = 1.0
    return phis, ccs, ind


_CACHE = {}


def kernel(x, S1, S2):
    x = np.asarray(x)
    if "k" not in _CACHE:
        phis, ccs, ind = make_constants(np.asarray(S1), np.asarray(S2))
        _CACHE["k"] = (build_nc(), phis, ccs, ind)
    nc, phis, ccs, ind = _CACHE["k"]

    xr = np.ascontiguousarray(x.reshape(B, C, P).astype(np.float32))
    in_maps = [
        {"x": xr, "phi": phis[j], "ind": ind, "cc": ccs[j]}
        for j in range(NCORES)
    ]
    res = run_bass_kernel_spmd(nc, in_maps, list(range(NCORES)))
    out = np.zeros((B, D), np.float32)
    for r in res.results:
        out += r["out"]
    return out.astype(x.dtype)



# revision 6
# speedup vs baseline: 1.6764x; 1.6764x over previous
"""Compact Bilinear Pooling (count-sketch + circular conv + spatial sum-pool)
as a Trainium2 Bass/Tile kernel, SPMD over 8 NeuronCores.

Math: with sk_i = flat @ S_i (flat: [B*P, C]), the reference computes
    out[b] = sum_{p in sample b} ifft( fft(sk1_p) * fft(sk2_p) ).real
Fold the (constant) sketch matrices into the DFT:  Phi_i = fft(S_i, axis=1),
so fft(sk_i,p) = x_p^T Phi_i.  Because sk are real, only the half spectrum
k = 0..D/2 is needed:
    Shat[b,k]  = sum_p (x_p^T Phi1[:,k]) * (x_p^T Phi2[:,k])
    out[b,d]   = sum_k wk*( Re Shat * cos(2 pi k d/D) - Im Shat * sin(...) )/D
Device pipeline per core (sharded by frequency: 512 of 4096 padded freqs),
all matmuls bf16 (PSUM accumulation in f32):
  1. Ghat tiles [128 pos, 512 freq] via bf16 matmuls.
  2. complex products on DVE -> bf16; per-sample position-reduction via
     transposed +-1 indicator matmuls (out [128 freq, 16 samp], free dim 16)
     accumulating Shat^T directly -- no DMA transpose needed.
  3. inverse DFT transposed: out [128 d, 16 samp] per d-tile (free dim 16)
     against cos/sin slabs already laid out [freq, d]; partial [8064, 16]
     per core; host sums the 8 partials and transposes.
"""

import numpy as np
import ml_dtypes

import concourse.bacc as bacc
import concourse.mybir as mybir
import concourse.tile as tile
from concourse.bass_utils import run_bass_kernel_spmd

# problem dims (hardcoded per spec)
B, C, H, W, D = 16, 512, 14, 14, 8000
P = H * W            # 196 positions per sample
BP = B * P           # 3136
KH = D // 2 + 1      # 4001 half-spectrum frequencies
KPAD = 4096          # padded to 8*512
NCORES = 8
KSL = KPAD // NCORES  # 512 freqs per core
NCC = C // 128        # 4 contraction chunks
NPT = (BP + 127) // 128  # 25 position tiles (24x128 + 64)
NKT = KSL // 128      # 4 k-chunks of the core's freq slice
DPAD = 8064           # 63 * 128 output-dim tiles (8000 padded)
NDT = DPAD // 128     # 63 d tiles
XSPLIT = 1024         # first x piece covers pts 0..7

F32 = mybir.dt.float32
BF16 = mybir.dt.bfloat16


def build_nc():
    nc = bacc.Bacc("TRN2", target_bir_lowering=False, debug=False)
    x_d = nc.dram_tensor("x", [C, BP], BF16, kind="ExternalInput")
    phi_d = nc.dram_tensor("phi", [128, NCC * 4 * KSL], BF16, kind="ExternalInput")
    ind_d = nc.dram_tensor("ind", [128, NPT * 2 * B], BF16, kind="ExternalInput")
    cc_d = nc.dram_tensor("cc", [2, NKT, 128, DPAD], BF16, kind="ExternalInput")
    out_d = nc.dram_tensor("out", [DPAD, B], F32, kind="ExternalOutput")

    with tile.TileContext(nc) as tc:
        with (
            tc.tile_pool(name="phi", bufs=1) as phi_pool,
            tc.tile_pool(name="xin", bufs=1) as x_pool,
            tc.tile_pool(name="ccs", bufs=1) as cc_pool,
            tc.tile_pool(name="bcp", bufs=4) as b_pool,
            tc.tile_pool(name="prd", bufs=8) as prod_pool,
            tc.tile_pool(name="sbf", bufs=1) as s_pool,
            tc.tile_pool(name="stage", bufs=2) as st_pool,
            tc.tile_pool(name="mm", bufs=6, space="PSUM") as mm_psum,
            tc.tile_pool(name="sac", bufs=1, space="PSUM") as s_psum,
            tc.tile_pool(name="inv", bufs=1, space="PSUM") as inv_psum,
        ):
            # ---- inputs, ordered so pt0 can start ASAP:
            # (phi cc, x cc first piece) pairs, then x second pieces, ind, cc
            phit, xta, xtb = [], [], []
            for cc in range(NCC):
                pt_ = phi_pool.tile([128, 4 * KSL], BF16, tag=f"phi{cc}")
                nc.sync.dma_start(pt_[:], phi_d.ap()[:, cc * 4 * KSL:(cc + 1) * 4 * KSL])
                phit.append(pt_)
                ta = x_pool.tile([128, XSPLIT], BF16, tag=f"xa{cc}")
                nc.sync.dma_start(ta[:], x_d.ap()[cc * 128:(cc + 1) * 128, 0:XSPLIT])
                xta.append(ta)
            for cc in range(NCC):
                tb = x_pool.tile([128, BP - XSPLIT], BF16, tag=f"xb{cc}")
                nc.sync.dma_start(tb[:], x_d.ap()[cc * 128:(cc + 1) * 128, XSPLIT:])
                xtb.append(tb)
            indt = phi_pool.tile([128, NPT * 2 * B], BF16, tag="ind")
            nc.sync.dma_start(indt[:], ind_d.ap())
            cct = {}
            for t in range(2):
                for kt in range(NKT):
                    ct = cc_pool.tile([128, DPAD], BF16, tag=f"cc{t}{kt}")
                    nc.sync.dma_start(ct[:], cc_d.ap()[t, kt])
                    cct[(t, kt)] = ct

            # ---- Shat^T accumulator: cols (kt, half, b); half 0=Re, 1=Im
            s_acc = s_psum.tile([128, NKT * 2 * B], F32, tag="sacc")

            # ---- main stage
            for pt in range(NPT):
                M = min(128, BP - pt * 128)
                if pt * 128 + M <= XSPLIT:
                    xs = [xta[cc][:, pt * 128:pt * 128 + M] for cc in range(NCC)]
                else:
                    o = pt * 128 - XSPLIT
                    xs = [xtb[cc][:, o:o + M] for cc in range(NCC)]
                g = [mm_psum.tile([128, KSL], F32, tag="mm", name=f"g{pt}_{m}")
                     for m in range(4)]
                for cc in range(NCC):
                    for m in range(4):  # 0:g1re 1:g1im 2:g2re 3:g2im
                        nc.tensor.matmul(
                            g[m][0:M, :],
                            lhsT=xs[cc],
                            rhs=phit[cc][:, m * KSL:(m + 1) * KSL],
                            start=(cc == 0),
                            stop=(cc == NCC - 1),
                        )
                b2re = b_pool.tile([128, KSL], F32, tag="b2re")
                nc.scalar.copy(b2re[0:M, :], g[2][0:M, :])
                b2im = b_pool.tile([128, KSL], F32, tag="b2im")
                nc.scalar.copy(b2im[0:M, :], g[3][0:M, :])

                prods = []
                for in0, in1 in ((g[0], b2re), (g[1], b2im),
                                 (g[0], b2im), (g[1], b2re)):
                    pr = prod_pool.tile([128, KSL], BF16, tag="prod")
                    nc.vector.tensor_mul(pr[0:M, :], in0[0:M, :], in1[0:M, :])
                    prods.append(pr)

                # per-sample reduce over positions, transposed:
                # s_acc[k, (kt,half,b)] += prod[p, k]^T @ ind[p, b]
                # i=0: RR(+)->Re  i=1: II(-)->Re  i=2: RI(+)->Im  i=3: IR(+)->Im
                for i, (pr, pat, half) in enumerate((
                    (prods[0], 0, 0), (prods[1], 1, 0),
                    (prods[2], 0, 1), (prods[3], 0, 1),
                )):
                    ic = (pt * 2 + pat) * B
                    for kt in range(NKT):
                        sc = kt * 2 * B + half * B
                        # one start/stop per PSUM bank: start=True zeroes the
                        # whole 2KB region, so only the first matmul into the
                        # bank may carry it
                        nc.tensor.matmul(
                            s_acc[:, sc:sc + B],
                            lhsT=pr[0:M, kt * 128:(kt + 1) * 128],
                            rhs=indt[0:M, ic:ic + B],
                            start=(pt == 0 and i == 0 and kt == 0),
                            stop=(pt == NPT - 1 and i == 3 and kt == NKT - 1),
                            skip_group_check=True,
                        )

            # ---- Shat^T -> bf16 SBUF (no transpose needed)
            sT = []
            for kt in range(NKT):
                t_ = s_pool.tile([128, 2 * B], BF16, tag=f"sT{kt}")
                nc.scalar.copy(t_[:], s_acc[:, kt * 2 * B:(kt + 1) * 2 * B])
                sT.append(t_)

            # ---- inverse DFT transposed: out[d, b] = sum_k C[k,d]*S[k,b]
            groups = [(0, 32), (32, 31)]
            for d0, nd in groups:
                pinv = inv_psum.tile([128, 512], F32, tag="inv")
                for i in range(nd):
                    dt = d0 + i
                    for idx in range(8):
                        t, kt = idx // 4, idx % 4
                        nc.tensor.matmul(
                            pinv[:, i * B:(i + 1) * B],
                            lhsT=cct[(t, kt)][:, dt * 128:(dt + 1) * 128],
                            rhs=sT[kt][:, t * B:(t + 1) * B],
                            start=(i == 0 and idx == 0),
                            stop=(i == nd - 1 and idx == 7),
                            skip_group_check=True,
                        )
                stage = st_pool.tile([128, 512], F32, tag="stage")
                nc.scalar.copy(stage[:, 0:nd * B], pinv[:, 0:nd * B])
                nc.sync.dma_start(
                    out_d.ap()[d0 * 128:(d0 + nd) * 128, :]
                         .rearrange("(dt p) b -> p dt b", p=128),
                    stage[:, 0:nd * B].rearrange("p (dt b) -> p dt b", b=B),
                )

    nc.compile()
    return nc


def make_constants(S1, S2):
    """Host-side constant prep from the sketch matrices (per-core slices)."""
    Phi = np.zeros((4, C, KPAD), np.float32)
    for i, S in enumerate((S1, S2)):
        F = np.fft.fft(S.astype(np.float64), axis=1)[:, :KH]
        Phi[2 * i, :, :KH] = F.real.astype(np.float32)
        Phi[2 * i + 1, :, :KH] = F.imag.astype(np.float32)

    k = np.arange(KPAD, dtype=np.float64)
    wk = np.where((k == 0) | (k == D // 2), 1.0, 2.0) / D
    wk[KH:] = 0.0
    ang = 2.0 * np.pi * np.outer(k, np.arange(D, dtype=np.float64)) / D
    Cst = np.zeros((2, KPAD, DPAD), np.float32)
    Cst[0, :, :D] = wk[:, None] * np.cos(ang)
    Cst[1, :, :D] = -wk[:, None] * np.sin(ang)
    Cst = Cst.astype(ml_dtypes.bfloat16)  # [2, KPAD, DPAD]

    # phi_packed[j]: [128, (cc, m, kk)] = Phi[m, cc*128+p, 512j+kk]
    arr = Phi.reshape(4, NCC, 128, NCORES, KSL)  # [m, cc, p, j, kk]
    phis, ccs = [], []
    for j in range(NCORES):
        a = arr[:, :, :, j]                      # [m, cc, p, kk]
        a = np.ascontiguousarray(np.transpose(a, (1, 0, 2, 3)))  # [cc, m, p, kk]
        phis.append(np.ascontiguousarray(
            a.transpose(2, 0, 1, 3).reshape(128, NCC * 4 * KSL)
        ).astype(ml_dtypes.bfloat16))
        c = Cst.reshape(2, NCORES, NKT, 128, DPAD)[:, j]  # [2, kt, 128, DPAD]
        ccs.append(np.ascontiguousarray(c))

    # indicators: [128, (pt, pat, b)]; pat 0 = +1, pat 1 = -1
    ind = np.zeros((128, NPT * 2 * B), np.float32)
    for pt in range(NPT):
        for r in range(min(128, BP - pt * 128)):
            b = (pt * 128 + r) // P
            ind[r, (pt * 2 + 0) * B + b] = 1.0
            ind[r, (pt * 2 + 1) * B + b] = -1.0
    return phis, ccs, ind.astype(ml_dtypes.bfloat16)


_CACHE = {}


def kernel(x, S1, S2):
    x = np.asarray(x)
    if "k" not in _CACHE:
        phis, ccs, ind = make_constants(np.asarray(S1), np.asarray(S2))
        _CACHE["k"] = (build_nc(), phis, ccs, ind)
    nc, phis, ccs, ind = _CACHE["k"]

    # [B, C, H, W] -> [C, B*P] bf16, row-contiguous for wide DMA lines
    xr = np.ascontiguousarray(
        x.reshape(B, C, P).transpose(1, 0, 2).reshape(C, BP)
    ).astype(ml_dtypes.bfloat16)
    in_maps = [
        {"x": xr, "phi": phis[j], "ind": ind, "cc": ccs[j]}
        for j in range(NCORES)
    ]
    res = run_bass_kernel_spmd(nc, in_maps, list(range(NCORES)))
    out = np.zeros((DPAD, B), np.float32)
    for r in res.results:
        out += r["out"]
    return np.ascontiguousarray(out[:D].T).astype(x.dtype)


# revision 16
# speedup vs baseline: 1.7030x; 1.0159x over previous
"""Compact Bilinear Pooling (count-sketch + circular conv + spatial sum-pool)
as a Trainium2 Bass/Tile kernel, SPMD over 8 NeuronCores.

Math: with sk_i = flat @ S_i (flat: [B*P, C]), the reference computes
    out[b] = sum_{p in sample b} ifft( fft(sk1_p) * fft(sk2_p) ).real
Fold the (constant) sketch matrices into the DFT:  Phi_i = fft(S_i, axis=1),
so fft(sk_i,p) = x_p^T Phi_i.  Because sk are real, only the half spectrum
k = 0..D/2 is needed:
    Shat[b,k]  = sum_p (x_p^T Phi1[:,k]) * (x_p^T Phi2[:,k])
    out[b,d]   = sum_k wk*( Re Shat * cos(2 pi k d/D) - Im Shat * sin(...) )/D
Device pipeline per core (sharded by frequency: 512 of 4096 padded freqs),
all matmuls bf16 (PSUM accumulation in f32):
  1. Ghat tiles [128 pos, 512 freq] via bf16 matmuls.
  2. complex products on DVE -> bf16; per-sample position-reduction via
     transposed +-1 indicator matmuls (out [128 freq, 16 samp], free dim 16)
     accumulating Shat^T directly -- no DMA transpose needed.
  3. inverse DFT transposed: out [128 d, 16 samp] per d-tile (free dim 16)
     against cos/sin slabs already laid out [freq, d]; partial [8064, 16]
     per core; host sums the 8 partials and transposes.
"""

import numpy as np
import ml_dtypes

import concourse.bacc as bacc
import concourse.mybir as mybir
import concourse.tile as tile
from concourse.bass_utils import run_bass_kernel_spmd

# problem dims (hardcoded per spec)
B, C, H, W, D = 16, 512, 14, 14, 8000
P = H * W            # 196 positions per sample
BP = B * P           # 3136
KH = D // 2 + 1      # 4001 half-spectrum frequencies
KPAD = 4096          # padded to 8*512
NCORES = 8
KSL = KPAD // NCORES  # 512 freqs per core
NCC = C // 128        # 4 contraction chunks
NPT = (BP + 127) // 128  # 25 position tiles (24x128 + 64)
NKT = KSL // 128      # 4 k-chunks of the core's freq slice
DPAD = 8064           # 63 * 128 output-dim tiles (8000 padded)
NDT = DPAD // 128     # 63 d tiles
XSPLIT = 1024         # first x piece covers pts 0..7

F32 = mybir.dt.float32
BF16 = mybir.dt.bfloat16


def build_nc():
    nc = bacc.Bacc("TRN2", target_bir_lowering=False, debug=False)
    x_d = nc.dram_tensor("x", [C, BP], BF16, kind="ExternalInput")
    phi_d = nc.dram_tensor("phi", [128, NCC * 4 * KSL], BF16, kind="ExternalInput")
    ind_d = nc.dram_tensor("ind", [128, NPT * 2 * B], BF16, kind="ExternalInput")
    cc_d = nc.dram_tensor("cc", [2, NKT, 128, DPAD], BF16, kind="ExternalInput")
    out_d = nc.dram_tensor("out", [DPAD, B], F32, kind="ExternalOutput")

    with tile.TileContext(nc) as tc:
        with (
            tc.tile_pool(name="phi", bufs=1) as phi_pool,
            tc.tile_pool(name="xin", bufs=1) as x_pool,
            tc.tile_pool(name="ccs", bufs=1) as cc_pool,
            tc.tile_pool(name="bcp", bufs=4) as b_pool,
            tc.tile_pool(name="prd", bufs=8) as prod_pool,
            tc.tile_pool(name="sbf", bufs=1) as s_pool,
            tc.tile_pool(name="stage", bufs=2) as st_pool,
            tc.tile_pool(name="mm", bufs=7, space="PSUM") as mm_psum,
            tc.tile_pool(name="sac", bufs=1, space="PSUM") as s_psum,
        ):
            # ---- inputs, ordered so pt0 can start ASAP: cc0's first phi
            # m-slice and pt0's x columns land first, then the rest in
            # cc-major order, then x tails / ind / inverse slabs
            phi0a = phi_pool.tile([128, KSL], BF16, tag="phi0a")
            nc.sync.dma_start(phi0a[:], phi_d.ap()[:, 0:KSL])
            x0a = x_pool.tile([128, 128], BF16, tag="x0a")
            nc.sync.dma_start(x0a[:], x_d.ap()[0:128, 0:128])
            phi0b = phi_pool.tile([128, 3 * KSL], BF16, tag="phi0b")
            nc.sync.dma_start(phi0b[:], phi_d.ap()[:, KSL:4 * KSL])
            x0b = x_pool.tile([128, XSPLIT - 128], BF16, tag="x0b")
            nc.sync.dma_start(x0b[:], x_d.ap()[0:128, 128:XSPLIT])
            phit, xta, xtb = [None], [None], []
            for cc in range(1, NCC):
                pt_ = phi_pool.tile([128, 4 * KSL], BF16, tag=f"phi{cc}")
                nc.sync.dma_start(pt_[:], phi_d.ap()[:, cc * 4 * KSL:(cc + 1) * 4 * KSL])
                phit.append(pt_)
                ta = x_pool.tile([128, XSPLIT], BF16, tag=f"xa{cc}")
                nc.sync.dma_start(ta[:], x_d.ap()[cc * 128:(cc + 1) * 128, 0:XSPLIT])
                xta.append(ta)
            for cc in range(NCC):
                tb = x_pool.tile([128, BP - XSPLIT], BF16, tag=f"xb{cc}")
                nc.sync.dma_start(tb[:], x_d.ap()[cc * 128:(cc + 1) * 128, XSPLIT:])
                xtb.append(tb)
            indt = phi_pool.tile([128, NPT * 2 * B], BF16, tag="ind")
            nc.sync.dma_start(indt[:], ind_d.ap())

            def phi_slice(cc, m):
                if cc == 0:
                    return phi0a[:] if m == 0 else phi0b[:, (m - 1) * KSL:m * KSL]
                return phit[cc][:, m * KSL:(m + 1) * KSL]

            def x_slice(cc, pt, M):
                if pt * 128 + M <= XSPLIT:
                    if cc == 0:
                        if pt == 0:
                            return x0a[:]
                        return x0b[:, (pt - 1) * 128:(pt - 1) * 128 + M]
                    return xta[cc][:, pt * 128:pt * 128 + M]
                o = pt * 128 - XSPLIT
                return xtb[cc][:, o:o + M]
            cct = {}
            for t in range(2):
                for kt in range(NKT):
                    ct = cc_pool.tile([128, DPAD], BF16, tag=f"cc{t}{kt}")
                    nc.sync.dma_start(ct[:], cc_d.ap()[t, kt])
                    cct[(t, kt)] = ct

            # ---- Shat^T accumulator: cols (kt, half, b); half 0=Re, 1=Im
            s_acc = s_psum.tile([128, NKT * 2 * B], F32, tag="sacc")

            # ---- main stage
            for pt in range(NPT):
                M = min(128, BP - pt * 128)
                g = [mm_psum.tile([128, KSL], F32, tag="mm", name=f"g{pt}_{m}")
                     for m in range(4)]
                for cc in range(NCC):
                    xs = x_slice(cc, pt, M)
                    for m in range(4):  # 0:g1re 1:g1im 2:g2re 3:g2im
                        nc.tensor.matmul(
                            g[m][0:M, :],
                            lhsT=xs,
                            rhs=phi_slice(cc, m),
                            start=(cc == 0),
                            stop=(cc == NCC - 1),
                        )
                # DVE can read only one PSUM operand per op: copy g2/g3 to
                # SBUF (ACT) and multiply against PSUM-resident g0/g1
                b2re = b_pool.tile([128, KSL], F32, tag="b2re")
                nc.scalar.copy(b2re[0:M, :], g[2][0:M, :])
                b2im = b_pool.tile([128, KSL], F32, tag="b2im")
                nc.scalar.copy(b2im[0:M, :], g[3][0:M, :])
                prods = []
                for in0, in1 in ((g[0], b2re), (g[1], b2im),
                                 (g[0], b2im), (g[1], b2re)):
                    pr = prod_pool.tile([128, KSL], BF16, tag="prod")
                    nc.vector.tensor_mul(pr[0:M, :], in0[0:M, :], in1[0:M, :])
                    prods.append(pr)

                # per-sample reduce over positions, transposed:
                # s_acc[k, (kt,half,b)] += prod[p, k]^T @ ind[p, b]
                # i=0: RR(+)->Re  i=1: II(-)->Re  i=2: RI(+)->Im  i=3: IR(+)->Im
                for i, (pr, pat, half) in enumerate((
                    (prods[0], 0, 0), (prods[1], 1, 0),
                    (prods[2], 0, 1), (prods[3], 0, 1),
                )):
                    ic = (pt * 2 + pat) * B
                    for kt in range(NKT):
                        sc = kt * 2 * B + half * B
                        # one start/stop per PSUM bank: start=True zeroes the
                        # whole 2KB region, so only the first matmul into the
                        # bank may carry it
                        nc.tensor.matmul(
                            s_acc[:, sc:sc + B],
                            lhsT=pr[0:M, kt * 128:(kt + 1) * 128],
                            rhs=indt[0:M, ic:ic + B],
                            start=(pt == 0 and i == 0 and kt == 0),
                            stop=(pt == NPT - 1 and i == 3 and kt == NKT - 1),
                            skip_group_check=True,
                        )

            # ---- Shat^T -> bf16 SBUF (no transpose needed); split ACT/DVE
            sT = []
            for kt in range(NKT):
                t_ = s_pool.tile([128, 2 * B], BF16, tag=f"sT{kt}")
                nc.scalar.copy(t_[:], s_acc[:, kt * 2 * B:(kt + 1) * 2 * B])
                sT.append(t_)

            # ---- inverse DFT transposed: out[d, b] = sum_k C[k,d]*S[k,b]
            # groups rotate through the mm pool's banks (stage 1 is done);
            # tiny last group keeps the copy+DMA+sem tail off the critical path
            groups = [(0, 21), (21, 21), (42, 20), (62, 1)]
            for d0, nd in groups:
                pinv = mm_psum.tile([128, nd * B], F32, tag="mm", name=f"pinv{d0}")
                for i in range(nd):
                    dt = d0 + i
                    for idx in range(8):
                        t, kt = idx // 4, idx % 4
                        nc.tensor.matmul(
                            pinv[:, i * B:(i + 1) * B],
                            lhsT=cct[(t, kt)][:, dt * 128:(dt + 1) * 128],
                            rhs=sT[kt][:, t * B:(t + 1) * B],
                            start=(i == 0 and idx == 0),
                            stop=(i == nd - 1 and idx == 7),
                            skip_group_check=True,
                        )
                stage = st_pool.tile([128, nd * B], F32, tag="stage", name=f"st{d0}")
                nc.scalar.copy(stage[:, 0:nd * B], pinv[:, 0:nd * B])
                nc.sync.dma_start(
                    out_d.ap()[d0 * 128:(d0 + nd) * 128, :]
                         .rearrange("(dt p) b -> p dt b", p=128),
                    stage[:, 0:nd * B].rearrange("p (dt b) -> p dt b", b=B),
                )

    nc.compile()
    return nc


def make_constants(S1, S2):
    """Host-side constant prep from the sketch matrices (per-core slices)."""
    Phi = np.zeros((4, C, KPAD), np.float32)
    for i, S in enumerate((S1, S2)):
        F = np.fft.fft(S.astype(np.float64), axis=1)[:, :KH]
        Phi[2 * i, :, :KH] = F.real.astype(np.float32)
        Phi[2 * i + 1, :, :KH] = F.imag.astype(np.float32)

    k = np.arange(KPAD, dtype=np.float64)
    wk = np.where((k == 0) | (k == D // 2), 1.0, 2.0) / D
    wk[KH:] = 0.0
    ang = 2.0 * np.pi * np.outer(k, np.arange(D, dtype=np.float64)) / D
    Cst = np.zeros((2, KPAD, DPAD), np.float32)
    Cst[0, :, :D] = wk[:, None] * np.cos(ang)
    Cst[1, :, :D] = -wk[:, None] * np.sin(ang)
    Cst = Cst.astype(ml_dtypes.bfloat16)  # [2, KPAD, DPAD]

    # phi_packed[j]: [128, (cc, m, kk)] = Phi[m, cc*128+p, 512j+kk]
    arr = Phi.reshape(4, NCC, 128, NCORES, KSL)  # [m, cc, p, j, kk]
    phis, ccs = [], []
    for j in range(NCORES):
        a = arr[:, :, :, j]                      # [m, cc, p, kk]
        a = np.ascontiguousarray(np.transpose(a, (1, 0, 2, 3)))  # [cc, m, p, kk]
        phis.append(np.ascontiguousarray(
            a.transpose(2, 0, 1, 3).reshape(128, NCC * 4 * KSL)
        ).astype(ml_dtypes.bfloat16))
        c = Cst.reshape(2, NCORES, NKT, 128, DPAD)[:, j]  # [2, kt, 128, DPAD]
        ccs.append(np.ascontiguousarray(c))

    # indicators: [128, (pt, pat, b)]; pat 0 = +1, pat 1 = -1
    ind = np.zeros((128, NPT * 2 * B), np.float32)
    for pt in range(NPT):
        for r in range(min(128, BP - pt * 128)):
            b = (pt * 128 + r) // P
            ind[r, (pt * 2 + 0) * B + b] = 1.0
            ind[r, (pt * 2 + 1) * B + b] = -1.0
    return phis, ccs, ind.astype(ml_dtypes.bfloat16)


_CACHE = {}


def kernel(x, S1, S2):
    x = np.asarray(x)
    if "k" not in _CACHE:
        phis, ccs, ind = make_constants(np.asarray(S1), np.asarray(S2))
        _CACHE["k"] = (build_nc(), phis, ccs, ind)
    nc, phis, ccs, ind = _CACHE["k"]

    # [B, C, H, W] -> [C, B*P] bf16, row-contiguous for wide DMA lines
    xr = np.ascontiguousarray(
        x.reshape(B, C, P).transpose(1, 0, 2).reshape(C, BP)
    ).astype(ml_dtypes.bfloat16)
    in_maps = [
        {"x": xr, "phi": phis[j], "ind": ind, "cc": ccs[j]}
        for j in range(NCORES)
    ]
    res = run_bass_kernel_spmd(nc, in_maps, list(range(NCORES)))
    out = np.zeros((DPAD, B), np.float32)
    for r in res.results:
        out += r["out"]
    return np.ascontiguousarray(out[:D].T).astype(x.dtype)


# revision 23
# speedup vs baseline: 1.7344x; 1.0184x over previous
"""Compact Bilinear Pooling (count-sketch + circular conv + spatial sum-pool)
as a Trainium2 Bass/Tile kernel, SPMD over 8 NeuronCores.

Math: with sk_i = flat @ S_i (flat: [B*P, C]), the reference computes
    out[b] = sum_{p in sample b} ifft( fft(sk1_p) * fft(sk2_p) ).real
Fold the (constant) sketch matrices into the DFT:  Phi_i = fft(S_i, axis=1),
so fft(sk_i,p) = x_p^T Phi_i.  Because sk are real, only the half spectrum
k = 0..D/2 is needed:
    Shat[b,k]  = sum_p (x_p^T Phi1[:,k]) * (x_p^T Phi2[:,k])
    out[b,d]   = sum_k wk*( Re Shat * cos(2 pi k d/D) - Im Shat * sin(...) )/D
Device pipeline per core (sharded by frequency: 512 of 4096 padded freqs),
all matmuls bf16 (PSUM accumulation in f32):
  1. Ghat tiles [128 pos, 512 freq] via bf16 matmuls.
  2. complex products on DVE -> bf16; per-sample position-reduction via
     transposed +-1 indicator matmuls (out [128 freq, 16 samp], free dim 16)
     accumulating Shat^T directly -- no DMA transpose needed.
  3. inverse DFT transposed: out [128 d, 16 samp] per d-tile (free dim 16)
     against cos/sin slabs already laid out [freq, d]; partial [8064, 16]
     per core; host sums the 8 partials and transposes.
"""

import numpy as np
import ml_dtypes

import concourse.bacc as bacc
import concourse.mybir as mybir
import concourse.tile as tile
from concourse.bass_utils import run_bass_kernel_spmd

# problem dims (hardcoded per spec)
B, C, H, W, D = 16, 512, 14, 14, 8000
P = H * W            # 196 positions per sample
BP = B * P           # 3136
KH = D // 2 + 1      # 4001 half-spectrum frequencies
KPAD = 4096          # padded to 8*512
NCORES = 8
KSL = KPAD // NCORES  # 512 freqs per core
NCC = C // 128        # 4 contraction chunks
NPT = (BP + 127) // 128  # 25 position tiles (24x128 + 64)
NKT = KSL // 128      # 4 k-chunks of the core's freq slice
DPAD = 8064           # 63 * 128 output-dim tiles (8000 padded)
NDT = DPAD // 128     # 63 d tiles
XSPLIT = 1024         # first x piece covers pts 0..7

F32 = mybir.dt.float32
BF16 = mybir.dt.bfloat16


def build_nc():
    nc = bacc.Bacc("TRN2", target_bir_lowering=False, debug=False)
    x_d = nc.dram_tensor("x", [C, BP], BF16, kind="ExternalInput")
    phi_d = nc.dram_tensor("phi", [128, NCC * 4 * KSL], BF16, kind="ExternalInput")
    ind_d = nc.dram_tensor("ind", [128, NPT * 2 * B], BF16, kind="ExternalInput")
    cc_d = nc.dram_tensor("cc", [2, NKT, 128, DPAD], BF16, kind="ExternalInput")
    out_d = nc.dram_tensor("out", [DPAD, B], F32, kind="ExternalOutput")

    with tile.TileContext(nc) as tc:
        with (
            tc.tile_pool(name="phi", bufs=1) as phi_pool,
            tc.tile_pool(name="xin", bufs=1) as x_pool,
            tc.tile_pool(name="ccs", bufs=1) as cc_pool,
            tc.tile_pool(name="bcp", bufs=5) as b_pool,
            tc.tile_pool(name="prd", bufs=6) as prod_pool,
            tc.tile_pool(name="sbf", bufs=1) as s_pool,
            tc.tile_pool(name="stage", bufs=4) as st_pool,
            tc.tile_pool(name="mm", bufs=7, space="PSUM") as mm_psum,
            tc.tile_pool(name="sac", bufs=1, space="PSUM") as s_psum,
        ):
            # ---- PE warmup: ramp the clock through its p-states on dummy
            # matmuls while the first input DMAs are in flight
            warm = phi_pool.tile([128, KSL], BF16, tag="warm")
            nc.vector.memset(warm[:], 0.0)
            wps = mm_psum.tile([128, KSL], F32, tag="mm", name="warmps")
            for w in range(8):
                nc.tensor.matmul(wps[:], lhsT=warm[:, 0:128], rhs=warm[:],
                                 start=True, stop=True, skip_group_check=True)

            # ---- inputs, ordered so pt0 can start ASAP: cc0's first phi
            # m-slice and pt0's x columns land first, then the rest in
            # cc-major order, then x tails / ind / inverse slabs
            phi0 = []
            for m in range(4):
                pm = phi_pool.tile([128, KSL], BF16, tag=f"phi0m{m}", name=f"phi0m{m}")
                nc.sync.dma_start(pm[:], phi_d.ap()[:, m * KSL:(m + 1) * KSL])
                phi0.append(pm)
                if m == 0:
                    x0a = x_pool.tile([128, 128], BF16, tag="x0a")
                    nc.sync.dma_start(x0a[:], x_d.ap()[0:128, 0:128])
            x0b = x_pool.tile([128, XSPLIT - 128], BF16, tag="x0b")
            nc.sync.dma_start(x0b[:], x_d.ap()[0:128, 128:XSPLIT])
            phit, xta, xtb = [None], [None], []
            for cc in range(1, NCC):
                pt_ = phi_pool.tile([128, 4 * KSL], BF16, tag=f"phi{cc}")
                nc.sync.dma_start(pt_[:], phi_d.ap()[:, cc * 4 * KSL:(cc + 1) * 4 * KSL])
                phit.append(pt_)
                ta = x_pool.tile([128, XSPLIT], BF16, tag=f"xa{cc}")
                nc.sync.dma_start(ta[:], x_d.ap()[cc * 128:(cc + 1) * 128, 0:XSPLIT])
                xta.append(ta)
            for cc in range(NCC):
                tb = x_pool.tile([128, BP - XSPLIT], BF16, tag=f"xb{cc}")
                nc.sync.dma_start(tb[:], x_d.ap()[cc * 128:(cc + 1) * 128, XSPLIT:])
                xtb.append(tb)
            indt = phi_pool.tile([128, NPT * 2 * B], BF16, tag="ind")
            nc.sync.dma_start(indt[:], ind_d.ap())

            def phi_slice(cc, m):
                if cc == 0:
                    return phi0[m][:]
                return phit[cc][:, m * KSL:(m + 1) * KSL]

            def x_slice(cc, pt, M):
                if pt * 128 + M <= XSPLIT:
                    if cc == 0:
                        if pt == 0:
                            return x0a[:]
                        return x0b[:, (pt - 1) * 128:(pt - 1) * 128 + M]
                    return xta[cc][:, pt * 128:pt * 128 + M]
                o = pt * 128 - XSPLIT
                return xtb[cc][:, o:o + M]
            cct = {}
            for t in range(2):
                for kt in range(NKT):
                    ct = cc_pool.tile([128, DPAD], BF16, tag=f"cc{t}{kt}")
                    nc.sync.dma_start(ct[:], cc_d.ap()[t, kt])
                    cct[(t, kt)] = ct

            # ---- Shat^T accumulator: cols (kt, half, b); half 0=Re, 1=Im
            s_acc = s_psum.tile([128, NKT * 2 * B], F32, tag="sacc")

            # ---- main stage
            for pt in range(NPT):
                M = min(128, BP - pt * 128)
                g = [mm_psum.tile([128, KSL], F32, tag="mm", name=f"g{pt}_{m}")
                     for m in range(4)]
                for cc in range(NCC):
                    xs = x_slice(cc, pt, M)
                    for m in range(4):  # 0:g1re 1:g1im 2:g2re 3:g2im
                        nc.tensor.matmul(
                            g[m][0:M, :],
                            lhsT=xs,
                            rhs=phi_slice(cc, m),
                            start=(cc == 0),
                            stop=(cc == NCC - 1),
                        )
                # copy all four g tiles PSUM->SBUF bf16 on ACT; the DVE
                # products then run all-SBUF/bf16 (fast mode) and the tail
                # drain after the last position tile shrinks
                gb = []
                for m in range(4):
                    t_ = b_pool.tile([128, KSL], BF16, tag=f"gb{m}",
                                     name=f"gb{pt}_{m}")
                    nc.scalar.copy(t_[0:M, :], g[m][0:M, :])
                    gb.append(t_)
                prods = []
                for in0, in1 in ((gb[0], gb[2]), (gb[1], gb[3]),
                                 (gb[0], gb[3]), (gb[1], gb[2])):
                    pr = prod_pool.tile([128, KSL], BF16, tag="prod")
                    nc.vector.tensor_mul(pr[0:M, :], in0[0:M, :], in1[0:M, :])
                    prods.append(pr)

                # per-sample reduce over positions, transposed:
                # s_acc[k, (kt,half,b)] += prod[p, k]^T @ ind[p, b]
                # i=0: RR(+)->Re  i=1: II(-)->Re  i=2: RI(+)->Im  i=3: IR(+)->Im
                for i, (pr, pat, half) in enumerate((
                    (prods[0], 0, 0), (prods[1], 1, 0),
                    (prods[2], 0, 1), (prods[3], 0, 1),
                )):
                    ic = (pt * 2 + pat) * B
                    for kt in range(NKT):
                        sc = kt * 2 * B + half * B
                        # one start/stop per PSUM bank: start=True zeroes the
                        # whole 2KB region, so only the first matmul into the
                        # bank may carry it
                        nc.tensor.matmul(
                            s_acc[:, sc:sc + B],
                            lhsT=pr[0:M, kt * 128:(kt + 1) * 128],
                            rhs=indt[0:M, ic:ic + B],
                            start=(pt == 0 and i == 0 and kt == 0),
                            stop=(pt == NPT - 1 and i == 3 and kt == NKT - 1),
                            skip_group_check=True,
                        )

            # ---- Shat^T -> bf16 SBUF (no transpose needed); split ACT/DVE
            sT = []
            for kt in range(NKT):
                t_ = s_pool.tile([128, 2 * B], BF16, tag=f"sT{kt}")
                nc.scalar.copy(t_[:], s_acc[:, kt * 2 * B:(kt + 1) * 2 * B])
                sT.append(t_)

            # ---- inverse DFT transposed: out[d, b] = sum_k C[k,d]*S[k,b]
            # groups rotate through the mm pool's banks (stage 1 is done);
            # tiny last group keeps the copy+DMA+sem tail off the critical path
            groups = [(0, 21), (21, 21), (42, 20), (62, 1)]
            for d0, nd in groups:
                pinv = mm_psum.tile([128, nd * B], F32, tag="mm", name=f"pinv{d0}")
                for i in range(nd):
                    dt = d0 + i
                    for idx in range(8):
                        t, kt = idx // 4, idx % 4
                        nc.tensor.matmul(
                            pinv[:, i * B:(i + 1) * B],
                            lhsT=cct[(t, kt)][:, dt * 128:(dt + 1) * 128],
                            rhs=sT[kt][:, t * B:(t + 1) * B],
                            start=(i == 0 and idx == 0),
                            stop=(i == nd - 1 and idx == 7),
                            skip_group_check=True,
                        )
                stage = st_pool.tile([128, nd * B], F32, tag="stage", name=f"st{d0}")
                nc.scalar.copy(stage[:, 0:nd * B], pinv[:, 0:nd * B])
                # spread the out-DMAs over different engine queues so their
                # dispatch does not serialize on the SP sequencer
                eng = {0: nc.sync, 21: nc.scalar, 42: nc.sync, 62: nc.scalar}[d0]
                eng.dma_start(
                    out_d.ap()[d0 * 128:(d0 + nd) * 128, :]
                         .rearrange("(dt p) b -> p dt b", p=128),
                    stage[:, 0:nd * B].rearrange("p (dt b) -> p dt b", b=B),
                )

    nc.compile()
    return nc


def make_constants(S1, S2):
    """Host-side constant prep from the sketch matrices (per-core slices)."""
    Phi = np.zeros((4, C, KPAD), np.float32)
    for i, S in enumerate((S1, S2)):
        F = np.fft.fft(S.astype(np.float64), axis=1)[:, :KH]
        Phi[2 * i, :, :KH] = F.real.astype(np.float32)
        Phi[2 * i + 1, :, :KH] = F.imag.astype(np.float32)

    k = np.arange(KPAD, dtype=np.float64)
    wk = np.where((k == 0) | (k == D // 2), 1.0, 2.0) / D
    wk[KH:] = 0.0
    ang = 2.0 * np.pi * np.outer(k, np.arange(D, dtype=np.float64)) / D
    Cst = np.zeros((2, KPAD, DPAD), np.float32)
    Cst[0, :, :D] = wk[:, None] * np.cos(ang)
    Cst[1, :, :D] = -wk[:, None] * np.sin(ang)
    Cst = Cst.astype(ml_dtypes.bfloat16)  # [2, KPAD, DPAD]

    # phi_packed[j]: [128, (cc, m, kk)] = Phi[m, cc*128+p, 512j+kk]
    arr = Phi.reshape(4, NCC, 128, NCORES, KSL)  # [m, cc, p, j, kk]
    phis, ccs = [], []
    for j in range(NCORES):
        a = arr[:, :, :, j]                      # [m, cc, p, kk]
        a = np.ascontiguousarray(np.transpose(a, (1, 0, 2, 3)))  # [cc, m, p, kk]
        phis.append(np.ascontiguousarray(
            a.transpose(2, 0, 1, 3).reshape(128, NCC * 4 * KSL)
        ).astype(ml_dtypes.bfloat16))
        c = Cst.reshape(2, NCORES, NKT, 128, DPAD)[:, j]  # [2, kt, 128, DPAD]
        ccs.append(np.ascontiguousarray(c))

    # indicators: [128, (pt, pat, b)]; pat 0 = +1, pat 1 = -1
    ind = np.zeros((128, NPT * 2 * B), np.float32)
    for pt in range(NPT):
        for r in range(min(128, BP - pt * 128)):
            b = (pt * 128 + r) // P
            ind[r, (pt * 2 + 0) * B + b] = 1.0
            ind[r, (pt * 2 + 1) * B + b] = -1.0
    return phis, ccs, ind.astype(ml_dtypes.bfloat16)


_CACHE = {}


def kernel(x, S1, S2):
    x = np.asarray(x)
    if "k" not in _CACHE:
        phis, ccs, ind = make_constants(np.asarray(S1), np.asarray(S2))
        _CACHE["k"] = (build_nc(), phis, ccs, ind)
    nc, phis, ccs, ind = _CACHE["k"]

    # [B, C, H, W] -> [C, B*P] bf16, row-contiguous for wide DMA lines
    xr = np.ascontiguousarray(
        x.reshape(B, C, P).transpose(1, 0, 2).reshape(C, BP)
    ).astype(ml_dtypes.bfloat16)
    in_maps = [
        {"x": xr, "phi": phis[j], "ind": ind, "cc": ccs[j]}
        for j in range(NCORES)
    ]
    res = run_bass_kernel_spmd(nc, in_maps, list(range(NCORES)))
    out = np.zeros((DPAD, B), np.float32)
    for r in res.results:
        out += r["out"]
    return np.ascontiguousarray(out[:D].T).astype(x.dtype)
